# revision 35
# baseline (speedup 1.0000x reference)
"""Trainium2 Bass kernel for a 6-layer GPT forward pass (B=4, T=1024, D=512,
H=8, HS=64, FF=2048, V=50257) on 8 NeuronCores.

Strategy (sequence-split body + vocab-split logits, pairwise collectives):
  - Host: embedding gather + weight re-layout/casting (bf16) + vocab padding
    + per-core causal masks.
  - Cores c and c+4 share batch c%4: core c owns the EVEN 128-token tiles
    {0,2,4,6}, core c+4 the ODD tiles {1,3,5,7} (interleaved split balances
    causal attention). Each core runs LN/QKV/attention/MLP for its 512 own
    tokens only; K/V for the full sequence arrive via one pairwise AllGather
    per layer (bit-exact bf16), read back into natural token order so the
    program is identical on all cores.
  - After the final LN, one more AllGather rebuilds the full-sequence
    activations; core c then computes vocab half c//4 for all 1024 tokens ->
    each core outputs [1024, 25216] bf16; host reassembles fp32.
  - Activations stay TRANSPOSED [D, tokens] so every matmul contracts on
    partitions; LN stats run on the PE from a bf16 shadow; row broadcasts
    use a bf16 e0-selector matmul (1 cycle/row) or the idle GPSIMD engine.
"""

import numpy as np
import ml_dtypes

import concourse.bass as bass
import concourse.bacc as bacc
import concourse.mybir as mybir
from concourse.bass import ts, ds
from concourse.tile import TileContext
from concourse.bass_utils import run_bass_kernel_spmd

# Prefer the combined ln+exp table set so Ln/Exp activations don't ping-pong
# ACT_TABLE_LOADs between per-function home sets (~1.3us per switch).
import concourse.hw_specs as _hw_specs
import concourse.bacc as _bacc_mod

_orig_get_tables = _hw_specs.get_activation_tables


def _tables_combined_first(module_arch):
    tabs = _orig_get_tables(module_arch)
    pref = "natural_log_exp_and_others"
    if pref not in tabs:
        return tabs
    excl = {AF.Exp, AF.Ln}
    return {k: (v if k == pref else (v - excl)) for k, v in tabs.items()}


AF = mybir.ActivationFunctionType
_bacc_mod.get_activation_tables = _tables_combined_first
F32 = mybir.dt.float32
BF16 = mybir.dt.bfloat16

P = 128
B, T, D, H, HS, FF, L, V = 4, 1024, 512, 8, 64, 2048, 6, 50257
DC = D // P            # 4 d-chunks
FC = FF // P           # 16 ff-chunks
NT = T // P            # 8 global token tiles of 128
TO = T // 2            # 512 own tokens per core
NTO = TO // P          # 4 own token tiles
NV = 25216             # per-core vocab cols (49*512 + 128); 2*NV = 50432 >= V
VPAD = 2 * NV
EPS = 1e-5
N_CORES = 8
CC_GROUPS = [[0, 4], [1, 5], [2, 6], [3, 7]]

bf16_np = ml_dtypes.bfloat16


# --------------------------------------------------------------------------
# device program
# --------------------------------------------------------------------------

def build_nc(n_layers=L):
    nc = bacc.Bacc()

    # ---------------- I/O ----------------
    x0_d = nc.dram_tensor("x0", [D, TO], F32, kind="ExternalInput")
    msk_d = nc.dram_tensor("msk", [P, 4, 1024], BF16, kind="ExternalInput")
    wq_d = nc.dram_tensor("wq", [n_layers, D, D], BF16, kind="ExternalInput")
    wk_d = nc.dram_tensor("wk", [n_layers, D, D], BF16, kind="ExternalInput")
    wv_d = nc.dram_tensor("wv", [n_layers, D, D], BF16, kind="ExternalInput")
    wp_d = nc.dram_tensor("wp", [n_layers, D, D], BF16, kind="ExternalInput")
    w1_d = nc.dram_tensor("w1", [n_layers, D, FF], BF16, kind="ExternalInput")
    w2_d = nc.dram_tensor("w2", [n_layers, FF, D], BF16, kind="ExternalInput")
    wlm_d = nc.dram_tensor("wlm", [D, NV], BF16, kind="ExternalInput")
    out_d = nc.dram_tensor("logits", [T, NV], BF16, kind="ExternalOutput")

    # ---------------- constants ----------------
    e0_np = np.zeros((P, P), dtype=bf16_np)
    e0_np[0, :] = 1.0
    e0_c = nc.inline_tensor(e0_np, name="e0sel")
    ones_bf_c = nc.inline_tensor(np.ones((P, 1), bf16_np), name="ones_b")

    with TileContext(nc) as tc:
        with tc.tile_pool(name="persist", bufs=1) as persist:
            # ---- persistent tiles (own-token activations are [.., TO]) ----
            x_sb = persist.tile([P, DC, TO], F32)          # residual x^T (own)
            xbf_sb = persist.tile([P, DC, TO], BF16)       # bf16 shadow of x
            ho_sb = persist.tile([P, DC, TO], BF16)        # LN output (own)
            q_sb = persist.tile([P, DC, TO], BF16)         # Q^T (pre-scaled)
            ko_sb = persist.tile([P, DC, TO], BF16)        # K^T own
            vo_sb = persist.tile([P, NTO, H, HS], BF16)    # V own (natural)
            k_sb = persist.tile([P, DC, T], BF16)          # K^T full (gathered)
            v_sb = persist.tile([P, NT, H, HS + 1], BF16)  # V' full + ones col
            ac_sb = persist.tile([P, DC, TO], BF16)        # attn-concat^T
            mid_sb = persist.tile([P, FC, TO], BF16)       # MLP mid^T
            hf_sb = persist.tile([P, DC, T], BF16)         # final LN, full seq
            mask_sb = persist.tile([P, 4, 1024], BF16)     # causal masks (in)
            e0_sb = persist.tile([P, P], BF16)
            # zeroed row bank for e0 broadcasts: row 0 = data, rows 1-127 = 0
            # slots: 0 rstd; 2 nmr
            rowbank = persist.tile([P, 4, 512], BF16)
            ones_b = persist.tile([P, 1], BF16)

            # ---- load constants / params / x0 ----
            nc.gpsimd.dma_start(mask_sb[:], msk_d[:])
            nc.gpsimd.dma_start(e0_sb[:], e0_c[:])
            nc.vector.memset(rowbank[:], 0.0)
            nc.gpsimd.dma_start(ones_b[:], ones_bf_c[:])
            nc.gpsimd.dma_start(
                x_sb[:], x0_d[:].rearrange("(c p) t -> p c t", p=P))
            for _c in range(DC):
                nc.vector.tensor_copy(xbf_sb[:, _c, :], x_sb[:, _c, :])

            # V' ones-column (written once; [:, :, :, :HS] rewritten per layer)
            nc.vector.memset(v_sb[:, :, :, HS], 1.0)

            with (
                tc.tile_pool(name="wqkv", bufs=2) as wqkv_pool,
                tc.tile_pool(name="w1p", bufs=1) as w1_pool,
                tc.tile_pool(name="w2p", bufs=1) as w2_pool,
                tc.tile_pool(name="tmp", bufs=2) as tmp_pool,
                tc.tile_pool(name="xsqp", bufs=1) as xsq_pool,
                tc.tile_pool(name="wei", bufs=4) as wei_pool,
                tc.tile_pool(name="rows", bufs=1) as row_pool,
                tc.tile_pool(name="rl", bufs=4) as rl_pool,
                tc.tile_pool(name="dram", bufs=2, space="DRAM") as dram_pool,
                # PSUM budget (8 banks): scr 2x[128,1024]=4 (scores, LN bc),
                # b1 4x one-bank tiles (stats, pa, linear/V pts)
                tc.tile_pool(name="ps_scr", bufs=2, space="PSUM") as ps_scr,
                tc.tile_pool(name="ps_b1", bufs=4, space="PSUM") as ps_b1,
            ):
                # ---- helpers ----
                def layer_norm(src_sb, dst_sb, ln_tag):
                    """src [P, DC, TO] f32 -> dst [P, DC, TO] bf16; LN over D.
                    gamma==1 / beta==0 (asserted host-side). Stats come from
                    the bf16 shadow so both stats matmuls run at bf16 rate."""
                    xsq = xsq_pool.tile([P, DC, TO], BF16, tag="xsq")
                    for c in range(DC):
                        nc.scalar.activation(
                            xsq[:, c, :], xbf_sb[:, c, :], AF.Square)
                    st = ps_b1.tile([33, 512], F32, tag="b1",
                                    name=f"st_{ln_tag}")
                    for c in range(DC):
                        nc.tensor.matmul(st[0:1, :], ones_b[:],
                                         xbf_sb[:, c, :],
                                         start=(c == 0), stop=(c == DC - 1))
                        nc.tensor.matmul(st[32:33, :], ones_b[:],
                                         xsq[:, c, :],
                                         start=(c == 0), stop=(c == DC - 1))
                    r_mun = row_pool.tile([1, 512], F32, tag="r_mun")
                    r_munb = row_pool.tile([1, 512], BF16, tag="r_munb")
                    r_mu2 = row_pool.tile([1, 512], F32, tag="r_mu2")
                    r_var = row_pool.tile([1, 512], F32, tag="r_var")
                    nc.vector.tensor_scalar_mul(r_mun[:], st[0:1, :], -1.0 / D)
                    nc.vector.tensor_scalar_mul(r_munb[:], st[0:1, :],
                                                -1.0 / D)
                    nc.vector.tensor_mul(r_mu2[:], r_mun[:], r_mun[:])
                    # var = (sumsq * 1/D) - mu^2   (fused) then +eps
                    nc.vector.scalar_tensor_tensor(
                        r_var[:], st[32:33, :], 1.0 / D, r_mu2[:],
                        mybir.AluOpType.mult, mybir.AluOpType.subtract)
                    nc.vector.tensor_scalar_add(r_var[:], r_var[:], EPS)
                    r_lnv = row_pool.tile([1, 512], F32, tag="r_lnv")
                    nc.scalar.activation(r_lnv[:], r_var[:], AF.Ln)
                    nc.scalar.activation(rowbank[0:1, 0, :], r_lnv[:], AF.Exp,
                                         scale=-0.5)
                    nc.vector.tensor_mul(rowbank[0:1, 2, :], r_munb[:],
                                         rowbank[0:1, 0, :])
                    bc = ps_scr.tile([P, 1024], F32, tag="scr")
                    nc.tensor.matmul(bc[:, 0:512], e0_sb[:], rowbank[:, 0, :],
                                     start=True, stop=True)
                    nc.tensor.matmul(bc[:, 512:1024], e0_sb[:],
                                     rowbank[:, 2, :],
                                     start=True, stop=True)
                    for c in range(DC):
                        tmp = tmp_pool.tile([P, 512], F32, tag="lnt")
                        nc.vector.tensor_mul(tmp[:], src_sb[:, c, :],
                                             bc[:, 0:512])
                        nc.vector.tensor_add(dst_sb[:, c, :], tmp[:],
                                             bc[:, 512:1024])

                def linear_T(w_sb, src_sb, M_chunks, K_chunks, evict):
                    for m in range(M_chunks):
                        pt = ps_b1.tile([P, 512], F32, tag="b1")
                        for c in range(K_chunks):
                            nc.tensor.matmul(pt[:], w_sb[:, c, ts(m, P)],
                                             src_sb[:, c, :],
                                             start=(c == 0),
                                             stop=(c == K_chunks - 1))
                        evict(pt, m)

                # ================= transformer layers =================
                for l in range(n_layers):
                    wq_sb = wqkv_pool.tile([P, DC, D], BF16, tag="wq")
                    wk_sb = wqkv_pool.tile([P, DC, D], BF16, tag="wk")
                    wv_sb = wqkv_pool.tile([P, DC, D], BF16, tag="wv")
                    wp_sb = wqkv_pool.tile([P, DC, D], BF16, tag="wp")
                    w1_sb = w1_pool.tile([P, DC, FF], BF16, tag="w1")
                    w2_sb = w2_pool.tile([P, FC, D], BF16, tag="w2")
                    nc.scalar.dma_start(
                        wq_sb[:], wq_d[l].rearrange("(c p) m -> p c m", p=P))
                    nc.scalar.dma_start(
                        wk_sb[:], wk_d[l].rearrange("(c p) m -> p c m", p=P))
                    nc.scalar.dma_start(
                        wv_sb[:], wv_d[l].rearrange("(c p) m -> p c m", p=P))
                    nc.scalar.dma_start(
                        wp_sb[:], wp_d[l].rearrange("(c p) m -> p c m", p=P))
                    nc.scalar.dma_start(
                        w1_sb[:], w1_d[l].rearrange("(c p) m -> p c m", p=P))
                    nc.scalar.dma_start(
                        w2_sb[:], w2_d[l].rearrange("(c p) m -> p c m", p=P))

                    # -- LN1 --
                    layer_norm(x_sb, ho_sb, f"ln1_{l}")

                    # -- K^T own; gather K first so scores wait only on it --
                    linear_T(wk_sb, ho_sb, DC, DC,
                             lambda pt, m: nc.vector.tensor_copy(
                                 ko_sb[:, m, :], pt[:]))
                    k_in = dram_pool.tile([P, 2048], BF16, tag="kin",
                                          name=f"kin{l}")
                    k_out = dram_pool.tile([2, P, 2048], BF16, tag="kout",
                                           name=f"kout{l}")
                    nc.gpsimd.dma_start(k_in[:],
                                        ko_sb[:].rearrange("p c t -> p (c t)"))
                    nc.gpsimd.collective_compute(
                        "AllGather", mybir.AluOpType.bypass,
                        replica_groups=CC_GROUPS,
                        ins=[k_in[:].opt()], outs=[k_out[:].opt()])

                    # -- V own (natural), second gather under the score phase
                    for tchunk in range(NTO):
                        pt = ps_b1.tile([P, 512], F32, tag="b1")
                        for c in range(DC):
                            nc.tensor.matmul(pt[:], ho_sb[:, c, ts(tchunk, P)],
                                             wv_sb[:, c, :],
                                             start=(c == 0), stop=(c == DC - 1))
                        nc.vector.tensor_copy(
                            vo_sb[:, tchunk, :, :],
                            pt[:].rearrange("p (h s) -> p h s", h=H))
                    v_in = dram_pool.tile([P, 2048], BF16, tag="vin",
                                          name=f"vin{l}")
                    v_out = dram_pool.tile([2, P, 2048], BF16, tag="vout",
                                           name=f"vout{l}")
                    nc.gpsimd.dma_start(v_in[:],
                                        vo_sb[:].rearrange("p n h s -> p (n h s)"))
                    nc.gpsimd.collective_compute(
                        "AllGather", mybir.AluOpType.bypass,
                        replica_groups=CC_GROUPS,
                        ins=[v_in[:].opt()], outs=[v_out[:].opt()])
                    # de-permute readback: rank r local tile i = global 2i+r
                    # (coalesced: one strided DMA per rank per tensor)
                    k_dst = k_sb[:].rearrange("p c (i r q) -> p r c i q",
                                              r=2, q=P)
                    for r in range(2):
                        kk = k_out[r].rearrange(
                            "p (c i q) -> p c i q", c=DC, i=NTO)
                        nc.sync.dma_start(k_dst[:, r], kk[:])
                        vv = v_out[r].rearrange(
                            "p (i h s) -> p i h s", i=NTO, h=H)
                        for i in range(NTO):
                            nc.sync.dma_start(v_sb[:, 2 * i + r, :, 0:HS],
                                              vv[:, i, :, :])

                    # -- Q^T own (overlaps the collective) --
                    linear_T(wq_sb, ho_sb, DC, DC,
                             lambda pt, m: nc.vector.tensor_copy(
                                 q_sb[:, m, :], pt[:]))

                    # -- attention: head-pair interleave, 2 tk-tiles per
                    # score tile (one EXP per [128,1024]), masks as data --
                    for hp in range(H // 2):
                        h0, h1 = 2 * hp, 2 * hp + 1
                        pa0 = ps_b1.tile([HS + 1, 512], F32, tag="b1",
                                         name=f"pa0_{l}_{hp}")
                        pa1 = ps_b1.tile([HS + 1, 512], F32, tag="b1",
                                         name=f"pa1_{l}_{hp}")
                        for kp in range(4):
                            weis = []
                            for idx in (0, 1):
                                off = 64 * idx
                                pscr = ps_scr.tile([P, 1024], F32, tag="scr")
                                for half in (0, 1):
                                    nc.tensor.matmul(
                                        pscr[:, ds(half * 512, 512)],
                                        k_sb[off:off + HS, hp,
                                             ts(2 * kp + half, P)],
                                        q_sb[off:off + HS, hp, :],
                                        start=True, stop=True)
                                wei = wei_pool.tile([P, 1024], BF16,
                                                    tag="wei")
                                nc.scalar.activation(wei[:], pscr[:], AF.Exp)
                                nc.vector.tensor_mul(wei[:], wei[:],
                                                     mask_sb[:, kp, :])
                                weis.append(wei)
                            for half in (0, 1):
                                kk = 2 * kp + half
                                hs_sl = ds(half * 512, 512)
                                nc.tensor.matmul(
                                    pa0[:], v_sb[:, kk, h0, :],
                                    weis[0][:, hs_sl],
                                    start=(kk == 0), stop=(kk == NT - 1))
                                nc.tensor.matmul(
                                    pa1[:], v_sb[:, kk, h1, :],
                                    weis[1][:, hs_sl],
                                    start=(kk == 0), stop=(kk == NT - 1))
                        for idx, pa in enumerate((pa0, pa1)):
                            off = 64 * idx
                            # 1/l = exp(-ln(l)) on scalar rows; broadcast on
                            # the idle GPSIMD engine (PE never stalls on it)
                            r_l = rl_pool.tile([1, 512], F32, tag="r_l")
                            nc.scalar.activation(
                                r_l[:], pa[HS:HS + 1, :], AF.Ln)
                            nc.scalar.activation(
                                r_l[:], r_l[:], AF.Exp, scale=-1.0)
                            rinv = tmp_pool.tile([64, 512], F32, tag="rinv")
                            nc.gpsimd.partition_broadcast(rinv[:], r_l[:])
                            nc.vector.tensor_mul(
                                ac_sb[off:off + HS, hp, :],
                                pa[0:HS, :], rinv[:])

                    def evict_resid(pt, m):
                        nc.vector.tensor_add(x_sb[:, m, :], x_sb[:, m, :],
                                             pt[:])
                        nc.vector.tensor_copy(xbf_sb[:, m, :], x_sb[:, m, :])

                    linear_T(wp_sb, ac_sb, DC, DC, evict_resid)

                    # -- LN2 --
                    layer_norm(x_sb, ho_sb, f"ln2_{l}")

                    # -- MLP --
                    def evict_mid(pt, m):
                        nc.scalar.activation(mid_sb[:, m, :], pt[:], AF.Relu)

                    linear_T(w1_sb, ho_sb, FC, DC, evict_mid)
                    linear_T(w2_sb, mid_sb, DC, FC, evict_resid)

                # ================= final LN + gather full sequence =========
                layer_norm(x_sb, ho_sb, "lnf")
                hf_in = dram_pool.tile([P, 2048], BF16, tag="kvin",
                                       name="hfin")
                hf_out = dram_pool.tile([2, P, 2048], BF16, tag="kvout",
                                        name="hfout")
                nc.gpsimd.dma_start(hf_in[:],
                                    ho_sb[:].rearrange("p c t -> p (c t)"))
                nc.gpsimd.collective_compute(
                    "AllGather", mybir.AluOpType.bypass,
                    replica_groups=CC_GROUPS,
                    ins=[hf_in[:].opt()], outs=[hf_out[:].opt()])
                hf_dst = hf_sb[:].rearrange("p c (i r q) -> p r c i q",
                                            r=2, q=P)
                for r in range(2):
                    hh = hf_out[r].rearrange("p (c i q) -> p c i q",
                                             c=DC, i=NTO)
                    nc.sync.dma_start(hf_dst[:, r], hh[:])

            # ================= logits (vocab-split, full sequence) =========
            with (
                tc.tile_pool(name="wlmp", bufs=2) as wlm_pool,
                tc.tile_pool(name="stage", bufs=3) as stage_pool,
                tc.tile_pool(name="ps_log", bufs=6, space="PSUM") as ps_log,
            ):
                GW = 3 * 512  # group width (cols); 3 PSUM banks/wave
                n_groups = (NV + GW - 1) // GW
                for g in range(n_groups):
                    g0 = g * GW
                    gw = min(GW, NV - g0)
                    wlm_sb = wlm_pool.tile([P, DC, GW], BF16, tag="wlm")
                    nc.scalar.dma_start(
                        wlm_sb[:, :, :gw],
                        wlm_d[:][:, g0:g0 + gw].rearrange(
                            "(c p) n -> p c n", p=P))
                    n_sub = (gw + 511) // 512
                    for m in range(NT):
                        st = stage_pool.tile([P, GW], BF16, tag="stage")
                        # c outer / n inner: the stationary hf tile (c, m)
                        # repeats across n
                        pts = [ps_log.tile([P, 512], F32, tag="log",
                                           name=f"pt{n}")
                               for n in range(n_sub)]
                        for c in range(DC):
                            for n in range(n_sub):
                                nw = min(512, gw - n * 512)
                                nc.tensor.matmul(
                                    pts[n][:, :nw],
                                    hf_sb[:, c, ts(m, P)],
                                    wlm_sb[:, c, ds(n * 512, nw)],
                                    start=(c == 0), stop=(c == DC - 1))
                        for n in range(n_sub):
                            nw = min(512, gw - n * 512)
                            if n % 2 == 0:
                                nc.scalar.copy(st[:, ds(n * 512, nw)],
                                               pts[n][:, :nw])
                            else:
                                nc.vector.tensor_copy(st[:, ds(n * 512, nw)],
                                                      pts[n][:, :nw])
                        nc.sync.dma_start(out_d[:][ts(m, P), g0:g0 + gw],
                                          st[:, :gw])

    nc.compile()
    return nc


# --------------------------------------------------------------------------
# host side
# --------------------------------------------------------------------------

_NC_CACHE = {}


def _get_nc(n_layers=L, debug=False):
    key = n_layers
    if key not in _NC_CACHE:
        _NC_CACHE[key] = build_nc(n_layers)
    return _NC_CACHE[key]


def _own_cols(rank):
    tiles = [2 * i + rank for i in range(NTO)]
    return np.concatenate([np.arange(t * P, (t + 1) * P) for t in tiles])


def _make_mask(rank):
    """[P, 4, 1024] bf16: mask[p, kp, half*512+tau] = 1 if causal-visible."""
    cols = _own_cols(rank)                       # global tq per local col
    m = np.zeros((P, 4, 1024), np.float32)
    for kp in range(4):
        for half in range(2):
            tk = (2 * kp + half) * P + np.arange(P)[:, None]
            m[:, kp, half * 512:(half + 1) * 512] = (tk <= cols[None, :])
    return m.astype(bf16_np)


def _prep_in_maps(index, tok_emb, pos_emb, Wq, Wk, Wv, Wproj, bproj,
                  ln1_g, ln1_b, ln2_g, ln2_b, W1, b1, W2, b2,
                  lnf_g, lnf_b, Wlm, n_layers=L):
    f32 = np.float32
    idx = np.asarray(index)
    tok = np.asarray(tok_emb, f32)
    pos = np.asarray(pos_emb, f32)
    x0 = tok[idx] + pos[None, :T]                       # [B, T, D]
    x0_t = np.ascontiguousarray(x0.transpose(0, 2, 1))  # [B, D, T]

    def to_bf(a):
        return np.ascontiguousarray(np.asarray(a, f32)[:n_layers]).astype(bf16_np)

    wq = np.asarray(Wq, f32)[:n_layers].transpose(0, 2, 1, 3).reshape(n_layers, D, D)
    wq = np.ascontiguousarray(wq * (HS ** -0.5)).astype(bf16_np)
    wk = np.ascontiguousarray(
        np.asarray(Wk, f32)[:n_layers].transpose(0, 2, 1, 3).reshape(n_layers, D, D)
    ).astype(bf16_np)
    wv = np.ascontiguousarray(
        np.asarray(Wv, f32)[:n_layers].transpose(0, 2, 1, 3).reshape(n_layers, D, D)
    ).astype(bf16_np)
    wp = to_bf(Wproj)
    w1 = to_bf(W1)
    w2 = to_bf(W2)
    wlm_pad = np.zeros((D, VPAD), f32)
    wlm_pad[:, :V] = np.asarray(Wlm, f32)
    wlm_bf = wlm_pad.astype(bf16_np)

    assert not np.any(np.asarray(bproj)) and not np.any(np.asarray(b1)) \
        and not np.any(np.asarray(b2)), "kernel assumes zero biases"
    for _g in (ln1_g, ln2_g):
        assert np.all(np.asarray(_g) == 1.0), "kernel assumes LN gamma == 1"
    for _b in (ln1_b, ln2_b):
        assert not np.any(np.asarray(_b)), "kernel assumes LN beta == 0"
    assert np.all(np.asarray(lnf_g) == 1.0) and not np.any(np.asarray(lnf_b))
    common = dict(wq=wq, wk=wk, wv=wv, wp=wp, w1=w1, w2=w2)
    masks = [_make_mask(0), _make_mask(1)]
    cols = [_own_cols(0), _own_cols(1)]
    in_maps = []
    for c in range(N_CORES):
        b = c % B
        rank = c // B
        m = dict(common)
        m["x0"] = np.ascontiguousarray(x0_t[b][:, cols[rank]])
        m["msk"] = masks[rank]
        m["wlm"] = np.ascontiguousarray(wlm_bf[:, rank * NV:(rank + 1) * NV])
        in_maps.append(m)
    return in_maps


def kernel(**inputs):
    nc = _get_nc()
    in_maps = _prep_in_maps(**inputs)
    res = run_bass_kernel_spmd(nc, in_maps, core_ids=list(range(N_CORES)))
    out = np.empty((B, T, V), np.float32)
    for b in range(B):
        lo = res.results[b]["logits"]          # vocab half 0
        hi = res.results[b + B]["logits"]      # vocab half 1
        out[b, :, :NV] = lo
        out[b, :, NV:] = hi[:, :V - NV]
    return out


# revision 36
# speedup vs baseline: 1.1672x; 1.1672x over previous
"""Trainium2 Bass kernel for a 6-layer GPT forward pass (B=4, T=1024, D=512,
H=8, HS=64, FF=2048, V=50257) on 8 NeuronCores.

Strategy (sequence-split body + vocab-split logits, pairwise collectives):
  - Host: embedding gather + weight re-layout/casting (bf16) + vocab padding
    + per-core causal masks.
  - Cores c and c+4 share batch c%4: core c owns the EVEN 128-token tiles
    {0,2,4,6}, core c+4 the ODD tiles {1,3,5,7} (interleaved split balances
    causal attention). Each core runs LN/QKV/attention/MLP for its 512 own
    tokens only; K/V for the full sequence arrive via one pairwise AllGather
    per layer (bit-exact bf16), read back into natural token order so the
    program is identical on all cores.
  - After the final LN, one more AllGather rebuilds the full-sequence
    activations; core c then computes vocab half c//4 for all 1024 tokens ->
    each core outputs [1024, 25216] bf16; host reassembles fp32.
  - Activations stay TRANSPOSED [D, tokens] so every matmul contracts on
    partitions; LN stats run on the PE from a bf16 shadow; row broadcasts
    use a bf16 e0-selector matmul (1 cycle/row) or the idle GPSIMD engine.
"""

import numpy as np
import ml_dtypes

import concourse.bass as bass
import concourse.bacc as bacc
import concourse.mybir as mybir
from concourse.bass import ts, ds
from concourse.tile import TileContext
from concourse.bass_utils import run_bass_kernel_spmd

# Prefer the combined ln+exp table set so Ln/Exp activations don't ping-pong
# ACT_TABLE_LOADs between per-function home sets (~1.3us per switch).
import concourse.hw_specs as _hw_specs
import concourse.bacc as _bacc_mod

_orig_get_tables = _hw_specs.get_activation_tables


def _tables_combined_first(module_arch):
    tabs = _orig_get_tables(module_arch)
    pref = "natural_log_exp_and_others"
    if pref not in tabs:
        return tabs
    excl = {AF.Exp, AF.Ln}
    return {k: (v if k == pref else (v - excl)) for k, v in tabs.items()}


AF = mybir.ActivationFunctionType
_bacc_mod.get_activation_tables = _tables_combined_first
F32 = mybir.dt.float32
BF16 = mybir.dt.bfloat16

P = 128
B, T, D, H, HS, FF, L, V = 4, 1024, 512, 8, 64, 2048, 6, 50257
DC = D // P            # 4 d-chunks
FC = FF // P           # 16 ff-chunks
NT = T // P            # 8 global token tiles of 128
TO = T // 2            # 512 own tokens per core
NTO = TO // P          # 4 own token tiles
NV = 25216             # per-core vocab cols (49*512 + 128); 2*NV = 50432 >= V
VPAD = 2 * NV
EPS = 1e-5
N_CORES = 8
CC_GROUPS = [[0, 4], [1, 5], [2, 6], [3, 7]]

bf16_np = ml_dtypes.bfloat16


# --------------------------------------------------------------------------
# device program
# --------------------------------------------------------------------------

def build_nc(n_layers=L):
    nc = bacc.Bacc()

    # ---------------- I/O ----------------
    x0_d = nc.dram_tensor("x0", [D, TO], F32, kind="ExternalInput")
    msk_d = nc.dram_tensor("msk", [P, 4, 1024], BF16, kind="ExternalInput")
    wq_d = nc.dram_tensor("wq", [n_layers, D, D], BF16, kind="ExternalInput")
    wk_d = nc.dram_tensor("wk", [n_layers, D, D], BF16, kind="ExternalInput")
    wv_d = nc.dram_tensor("wv", [n_layers, D, D], BF16, kind="ExternalInput")
    wp_d = nc.dram_tensor("wp", [n_layers, D, D], BF16, kind="ExternalInput")
    w1_d = nc.dram_tensor("w1", [n_layers, D, FF], BF16, kind="ExternalInput")
    w2_d = nc.dram_tensor("w2", [n_layers, FF, D], BF16, kind="ExternalInput")
    wlm_d = nc.dram_tensor("wlm", [D, NV], BF16, kind="ExternalInput")
    out_d = nc.dram_tensor("logits", [T, NV], BF16, kind="ExternalOutput")

    # ---------------- constants ----------------
    e0_np = np.zeros((P, P), dtype=bf16_np)
    e0_np[0, :] = 1.0
    e0_c = nc.inline_tensor(e0_np, name="e0sel")
    ones_bf_c = nc.inline_tensor(np.ones((P, 1), bf16_np), name="ones_b")

    with TileContext(nc) as tc:
        with tc.tile_pool(name="persist", bufs=1) as persist:
            # ---- persistent tiles (own-token activations are [.., TO]) ----
            x_sb = persist.tile([P, DC, TO], F32)          # residual x^T (own)
            xbf_sb = persist.tile([P, DC, TO], BF16)       # bf16 shadow of x
            ho_sb = persist.tile([P, DC, TO], BF16)        # LN output (own)
            q_sb = persist.tile([P, DC, TO], BF16)         # Q^T (pre-scaled)
            ko_sb = persist.tile([P, DC, TO], BF16)        # K^T own
            vo_sb = persist.tile([P, NTO, H, HS], BF16)    # V own (natural)
            k_sb = persist.tile([P, DC, T], BF16)          # K^T full (gathered)
            v_sb = persist.tile([P, NT, H, HS + 1], BF16)  # V' full + ones col
            ac_sb = persist.tile([P, DC, TO], BF16)        # attn-concat^T
            mid_sb = persist.tile([P, FC, TO], BF16)       # MLP mid^T
            hf_sb = persist.tile([P, DC, T], BF16)         # final LN, full seq
            mask_sb = persist.tile([P, 4, 1024], BF16)     # causal masks (in)
            e0_sb = persist.tile([P, P], BF16)
            # zeroed row bank for e0 broadcasts: row 0 = data, rows 1-127 = 0
            # slots: 0 rstd; 2 nmr
            rowbank = persist.tile([P, 4, 512], BF16)
            ones_b = persist.tile([P, 1], BF16)

            # ---- load constants / params / x0 ----
            nc.gpsimd.dma_start(mask_sb[:], msk_d[:])
            nc.gpsimd.dma_start(e0_sb[:], e0_c[:])
            nc.vector.memset(rowbank[:], 0.0)
            nc.gpsimd.dma_start(ones_b[:], ones_bf_c[:])
            nc.gpsimd.dma_start(
                x_sb[:], x0_d[:].rearrange("(c p) t -> p c t", p=P))
            for _c in range(DC):
                nc.vector.tensor_copy(xbf_sb[:, _c, :], x_sb[:, _c, :])

            # V' ones-column (written once; [:, :, :, :HS] rewritten per layer)
            nc.vector.memset(v_sb[:, :, :, HS], 1.0)

            with (
                tc.tile_pool(name="wqkv", bufs=2) as wqkv_pool,
                tc.tile_pool(name="w1p", bufs=1) as w1_pool,
                tc.tile_pool(name="w2p", bufs=1) as w2_pool,
                tc.tile_pool(name="tmp", bufs=2) as tmp_pool,
                tc.tile_pool(name="xsqp", bufs=1) as xsq_pool,
                tc.tile_pool(name="wei", bufs=4) as wei_pool,
                tc.tile_pool(name="rows", bufs=1) as row_pool,
                tc.tile_pool(name="rl", bufs=4) as rl_pool,
                tc.tile_pool(name="dram", bufs=2, space="DRAM") as dram_pool,
                # PSUM budget (8 banks): scr 2x[128,1024]=4 (scores, LN bc),
                # b1 4x one-bank tiles (stats, pa, linear/V pts)
                tc.tile_pool(name="ps_scr", bufs=2, space="PSUM") as ps_scr,
                tc.tile_pool(name="ps_b1", bufs=4, space="PSUM") as ps_b1,
            ):
                # ---- helpers ----
                def layer_norm(src_sb, dst_sb, ln_tag):
                    """src [P, DC, TO] f32 -> dst [P, DC, TO] bf16; LN over D.
                    gamma==1 / beta==0 (asserted host-side). Stats come from
                    the bf16 shadow so both stats matmuls run at bf16 rate."""
                    xsq = xsq_pool.tile([P, DC, TO], BF16, tag="xsq")
                    for c in range(DC):
                        nc.scalar.activation(
                            xsq[:, c, :], xbf_sb[:, c, :], AF.Square)
                    st = ps_b1.tile([33, 512], F32, tag="b1",
                                    name=f"st_{ln_tag}")
                    for c in range(DC):
                        nc.tensor.matmul(st[0:1, :], ones_b[:],
                                         xbf_sb[:, c, :],
                                         start=(c == 0), stop=(c == DC - 1))
                        nc.tensor.matmul(st[32:33, :], ones_b[:],
                                         xsq[:, c, :],
                                         start=(c == 0), stop=(c == DC - 1))
                    r_mun = row_pool.tile([1, 512], F32, tag="r_mun")
                    r_munb = row_pool.tile([1, 512], BF16, tag="r_munb")
                    r_mu2 = row_pool.tile([1, 512], F32, tag="r_mu2")
                    r_var = row_pool.tile([1, 512], F32, tag="r_var")
                    nc.vector.tensor_scalar_mul(r_mun[:], st[0:1, :], -1.0 / D)
                    nc.vector.tensor_scalar_mul(r_munb[:], st[0:1, :],
                                                -1.0 / D)
                    nc.vector.tensor_mul(r_mu2[:], r_mun[:], r_mun[:])
                    # var = (sumsq * 1/D) - mu^2   (fused) then +eps
                    nc.vector.scalar_tensor_tensor(
                        r_var[:], st[32:33, :], 1.0 / D, r_mu2[:],
                        mybir.AluOpType.mult, mybir.AluOpType.subtract)
                    nc.vector.tensor_scalar_add(r_var[:], r_var[:], EPS)
                    r_lnv = row_pool.tile([1, 512], F32, tag="r_lnv")
                    nc.scalar.activation(r_lnv[:], r_var[:], AF.Ln)
                    nc.scalar.activation(rowbank[0:1, 0, :], r_lnv[:], AF.Exp,
                                         scale=-0.5)
                    nc.vector.tensor_mul(rowbank[0:1, 2, :], r_munb[:],
                                         rowbank[0:1, 0, :])
                    bc = ps_scr.tile([P, 1024], F32, tag="scr")
                    nc.tensor.matmul(bc[:, 0:512], e0_sb[:], rowbank[:, 0, :],
                                     start=True, stop=True)
                    nc.tensor.matmul(bc[:, 512:1024], e0_sb[:],
                                     rowbank[:, 2, :],
                                     start=True, stop=True)
                    for c in range(DC):
                        tmp = tmp_pool.tile([P, 512], F32, tag="lnt")
                        nc.vector.tensor_mul(tmp[:], src_sb[:, c, :],
                                             bc[:, 0:512])
                        nc.vector.tensor_add(dst_sb[:, c, :], tmp[:],
                                             bc[:, 512:1024])

                def linear_T(w_sb, src_sb, M_chunks, K_chunks, evict):
                    for m in range(M_chunks):
                        pt = ps_b1.tile([P, 512], F32, tag="b1")
                        for c in range(K_chunks):
                            nc.tensor.matmul(pt[:], w_sb[:, c, ts(m, P)],
                                             src_sb[:, c, :],
                                             start=(c == 0),
                                             stop=(c == K_chunks - 1))
                        evict(pt, m)

                # ================= transformer layers =================
                for l in range(n_layers):
                    wq_sb = wqkv_pool.tile([P, DC, D], BF16, tag="wq")
                    wk_sb = wqkv_pool.tile([P, DC, D], BF16, tag="wk")
                    wv_sb = wqkv_pool.tile([P, DC, D], BF16, tag="wv")
                    wp_sb = wqkv_pool.tile([P, DC, D], BF16, tag="wp")
                    w1_sb = w1_pool.tile([P, DC, FF], BF16, tag="w1")
                    w2_sb = w2_pool.tile([P, FC, D], BF16, tag="w2")
                    nc.gpsimd.dma_start(
                        wq_sb[:], wq_d[l].rearrange("(c p) m -> p c m", p=P))
                    nc.gpsimd.dma_start(
                        wk_sb[:], wk_d[l].rearrange("(c p) m -> p c m", p=P))
                    nc.gpsimd.dma_start(
                        wv_sb[:], wv_d[l].rearrange("(c p) m -> p c m", p=P))
                    nc.gpsimd.dma_start(
                        wp_sb[:], wp_d[l].rearrange("(c p) m -> p c m", p=P))
                    nc.gpsimd.dma_start(
                        w1_sb[:], w1_d[l].rearrange("(c p) m -> p c m", p=P))
                    nc.gpsimd.dma_start(
                        w2_sb[:], w2_d[l].rearrange("(c p) m -> p c m", p=P))

                    # -- LN1 --
                    layer_norm(x_sb, ho_sb, f"ln1_{l}")

                    # -- K^T own; gather K first so scores wait only on it --
                    linear_T(wk_sb, ho_sb, DC, DC,
                             lambda pt, m: nc.vector.tensor_copy(
                                 ko_sb[:, m, :], pt[:]))
                    k_in = dram_pool.tile([P, 2048], BF16, tag="kin",
                                          name=f"kin{l}")
                    k_out = dram_pool.tile([2, P, 2048], BF16, tag="kout",
                                           name=f"kout{l}")
                    nc.gpsimd.dma_start(k_in[:],
                                        ko_sb[:].rearrange("p c t -> p (c t)"))
                    nc.gpsimd.collective_compute(
                        "AllGather", mybir.AluOpType.bypass,
                        replica_groups=CC_GROUPS,
                        ins=[k_in[:].opt()], outs=[k_out[:].opt()])

                    # -- V own (natural), second gather under the score phase
                    for tchunk in range(NTO):
                        pt = ps_b1.tile([P, 512], F32, tag="b1")
                        for c in range(DC):
                            nc.tensor.matmul(pt[:], ho_sb[:, c, ts(tchunk, P)],
                                             wv_sb[:, c, :],
                                             start=(c == 0), stop=(c == DC - 1))
                        nc.vector.tensor_copy(
                            vo_sb[:, tchunk, :, :],
                            pt[:].rearrange("p (h s) -> p h s", h=H))
                    v_in = dram_pool.tile([P, 2048], BF16, tag="vin",
                                          name=f"vin{l}")
                    v_out = dram_pool.tile([2, P, 2048], BF16, tag="vout",
                                           name=f"vout{l}")
                    nc.gpsimd.dma_start(v_in[:],
                                        vo_sb[:].rearrange("p n h s -> p (n h s)"))
                    nc.gpsimd.collective_compute(
                        "AllGather", mybir.AluOpType.bypass,
                        replica_groups=CC_GROUPS,
                        ins=[v_in[:].opt()], outs=[v_out[:].opt()])
                    # de-permute readback: rank r local tile i = global 2i+r
                    # (coalesced: one strided DMA per rank per tensor)
                    k_dst = k_sb[:].rearrange("p c (i r q) -> p r c i q",
                                              r=2, q=P)
                    for r in range(2):
                        kk = k_out[r].rearrange(
                            "p (c i q) -> p c i q", c=DC, i=NTO)
                        nc.sync.dma_start(k_dst[:, r], kk[:])
                        vv = v_out[r].rearrange(
                            "p (i h s) -> p i h s", i=NTO, h=H)
                        for i in range(NTO):
                            nc.sync.dma_start(v_sb[:, 2 * i + r, :, 0:HS],
                                              vv[:, i, :, :])

                    # -- Q^T own (overlaps the collective) --
                    linear_T(wq_sb, ho_sb, DC, DC,
                             lambda pt, m: nc.vector.tensor_copy(
                                 q_sb[:, m, :], pt[:]))

                    # -- attention: head-pair interleave, 2 tk-tiles per
                    # score tile (one EXP per [128,1024]), masks as data --
                    for hp in range(H // 2):
                        h0, h1 = 2 * hp, 2 * hp + 1
                        pa0 = ps_b1.tile([HS + 1, 512], F32, tag="b1",
                                         name=f"pa0_{l}_{hp}")
                        pa1 = ps_b1.tile([HS + 1, 512], F32, tag="b1",
                                         name=f"pa1_{l}_{hp}")
                        for kp in range(4):
                            weis = []
                            for idx in (0, 1):
                                off = 64 * idx
                                pscr = ps_scr.tile([P, 1024], F32, tag="scr")
                                for half in (0, 1):
                                    nc.tensor.matmul(
                                        pscr[:, ds(half * 512, 512)],
                                        k_sb[off:off + HS, hp,
                                             ts(2 * kp + half, P)],
                                        q_sb[off:off + HS, hp, :],
                                        start=True, stop=True)
                                wei = wei_pool.tile([P, 1024], BF16,
                                                    tag="wei")
                                nc.scalar.activation(wei[:], pscr[:], AF.Exp)
                                nc.vector.tensor_mul(wei[:], wei[:],
                                                     mask_sb[:, kp, :])
                                weis.append(wei)
                            for half in (0, 1):
                                kk = 2 * kp + half
                                hs_sl = ds(half * 512, 512)
                                nc.tensor.matmul(
                                    pa0[:], v_sb[:, kk, h0, :],
                                    weis[0][:, hs_sl],
                                    start=(kk == 0), stop=(kk == NT - 1))
                                nc.tensor.matmul(
                                    pa1[:], v_sb[:, kk, h1, :],
                                    weis[1][:, hs_sl],
                                    start=(kk == 0), stop=(kk == NT - 1))
                        for idx, pa in enumerate((pa0, pa1)):
                            off = 64 * idx
                            # 1/l = exp(-ln(l)) on scalar rows; broadcast on
                            # the idle GPSIMD engine (PE never stalls on it)
                            r_l = rl_pool.tile([1, 512], F32, tag="r_l")
                            nc.scalar.activation(
                                r_l[:], pa[HS:HS + 1, :], AF.Ln)
                            nc.scalar.activation(
                                r_l[:], r_l[:], AF.Exp, scale=-1.0)
                            rinv = tmp_pool.tile([64, 512], F32, tag="rinv")
                            nc.gpsimd.partition_broadcast(rinv[:], r_l[:])
                            nc.vector.tensor_mul(
                                ac_sb[off:off + HS, hp, :],
                                pa[0:HS, :], rinv[:])

                    def evict_resid(pt, m):
                        nc.vector.tensor_add(x_sb[:, m, :], x_sb[:, m, :],
                                             pt[:])
                        nc.vector.tensor_copy(xbf_sb[:, m, :], x_sb[:, m, :])

                    linear_T(wp_sb, ac_sb, DC, DC, evict_resid)

                    # -- LN2 --
                    layer_norm(x_sb, ho_sb, f"ln2_{l}")

                    # -- MLP --
                    def evict_mid(pt, m):
                        nc.scalar.activation(mid_sb[:, m, :], pt[:], AF.Relu)

                    linear_T(w1_sb, ho_sb, FC, DC, evict_mid)
                    linear_T(w2_sb, mid_sb, DC, FC, evict_resid)

                # ================= final LN + gather full sequence =========
                layer_norm(x_sb, ho_sb, "lnf")
                hf_in = dram_pool.tile([P, 2048], BF16, tag="kvin",
                                       name="hfin")
                hf_out = dram_pool.tile([2, P, 2048], BF16, tag="kvout",
                                        name="hfout")
                nc.gpsimd.dma_start(hf_in[:],
                                    ho_sb[:].rearrange("p c t -> p (c t)"))
                nc.gpsimd.collective_compute(
                    "AllGather", mybir.AluOpType.bypass,
                    replica_groups=CC_GROUPS,
                    ins=[hf_in[:].opt()], outs=[hf_out[:].opt()])
                hf_dst = hf_sb[:].rearrange("p c (i r q) -> p r c i q",
                                            r=2, q=P)
                for r in range(2):
                    hh = hf_out[r].rearrange("p (c i q) -> p c i q",
                                             c=DC, i=NTO)
                    nc.sync.dma_start(hf_dst[:, r], hh[:])

            # ================= logits (vocab-split, full sequence) =========
            with (
                tc.tile_pool(name="wlmp", bufs=2) as wlm_pool,
                tc.tile_pool(name="stage", bufs=3) as stage_pool,
                tc.tile_pool(name="ps_log", bufs=6, space="PSUM") as ps_log,
            ):
                GW = 6 * 512  # group width (cols)
                n_groups = (NV + GW - 1) // GW
                for g in range(n_groups):
                    g0 = g * GW
                    gw = min(GW, NV - g0)
                    wlm_sb = wlm_pool.tile([P, DC, GW], BF16, tag="wlm")
                    nc.gpsimd.dma_start(
                        wlm_sb[:, :, :gw],
                        wlm_d[:][:, g0:g0 + gw].rearrange(
                            "(c p) n -> p c n", p=P))
                    n_sub = (gw + 511) // 512
                    for m in range(NT):
                        st = stage_pool.tile([P, GW], BF16, tag="stage")
                        # c outer / n inner: the stationary hf tile (c, m)
                        # repeats across n
                        pts = [ps_log.tile([P, 512], F32, tag="log",
                                           name=f"pt{n}")
                               for n in range(n_sub)]
                        for c in range(DC):
                            for n in range(n_sub):
                                nw = min(512, gw - n * 512)
                                nc.tensor.matmul(
                                    pts[n][:, :nw],
                                    hf_sb[:, c, ts(m, P)],
                                    wlm_sb[:, c, ds(n * 512, nw)],
                                    start=(c == 0), stop=(c == DC - 1))
                        for n in range(n_sub):
                            nw = min(512, gw - n * 512)
                            if n % 2 == 0:
                                nc.scalar.copy(st[:, ds(n * 512, nw)],
                                               pts[n][:, :nw])
                            else:
                                nc.vector.tensor_copy(st[:, ds(n * 512, nw)],
                                                      pts[n][:, :nw])
                        nc.sync.dma_start(out_d[:][ts(m, P), g0:g0 + gw],
                                          st[:, :gw])

    nc.compile()
    return nc


# --------------------------------------------------------------------------
# host side
# --------------------------------------------------------------------------

_NC_CACHE = {}


def _get_nc(n_layers=L, debug=False):
    key = n_layers
    if key not in _NC_CACHE:
        _NC_CACHE[key] = build_nc(n_layers)
    return _NC_CACHE[key]


def _own_cols(rank):
    tiles = [2 * i + rank for i in range(NTO)]
    return np.concatenate([np.arange(t * P, (t + 1) * P) for t in tiles])


def _make_mask(rank):
    """[P, 4, 1024] bf16: mask[p, kp, half*512+tau] = 1 if causal-visible."""
    cols = _own_cols(rank)                       # global tq per local col
    m = np.zeros((P, 4, 1024), np.float32)
    for kp in range(4):
        for half in range(2):
            tk = (2 * kp + half) * P + np.arange(P)[:, None]
            m[:, kp, half * 512:(half + 1) * 512] = (tk <= cols[None, :])
    return m.astype(bf16_np)


def _prep_in_maps(index, tok_emb, pos_emb, Wq, Wk, Wv, Wproj, bproj,
                  ln1_g, ln1_b, ln2_g, ln2_b, W1, b1, W2, b2,
                  lnf_g, lnf_b, Wlm, n_layers=L):
    f32 = np.float32
    idx = np.asarray(index)
    tok = np.asarray(tok_emb, f32)
    pos = np.asarray(pos_emb, f32)
    x0 = tok[idx] + pos[None, :T]                       # [B, T, D]
    x0_t = np.ascontiguousarray(x0.transpose(0, 2, 1))  # [B, D, T]

    def to_bf(a):
        return np.ascontiguousarray(np.asarray(a, f32)[:n_layers]).astype(bf16_np)

    wq = np.asarray(Wq, f32)[:n_layers].transpose(0, 2, 1, 3).reshape(n_layers, D, D)
    wq = np.ascontiguousarray(wq * (HS ** -0.5)).astype(bf16_np)
    wk = np.ascontiguousarray(
        np.asarray(Wk, f32)[:n_layers].transpose(0, 2, 1, 3).reshape(n_layers, D, D)
    ).astype(bf16_np)
    wv = np.ascontiguousarray(
        np.asarray(Wv, f32)[:n_layers].transpose(0, 2, 1, 3).reshape(n_layers, D, D)
    ).astype(bf16_np)
    wp = to_bf(Wproj)
    w1 = to_bf(W1)
    w2 = to_bf(W2)
    wlm_pad = np.zeros((D, VPAD), f32)
    wlm_pad[:, :V] = np.asarray(Wlm, f32)
    wlm_bf = wlm_pad.astype(bf16_np)

    assert not np.any(np.asarray(bproj)) and not np.any(np.asarray(b1)) \
        and not np.any(np.asarray(b2)), "kernel assumes zero biases"
    for _g in (ln1_g, ln2_g):
        assert np.all(np.asarray(_g) == 1.0), "kernel assumes LN gamma == 1"
    for _b in (ln1_b, ln2_b):
        assert not np.any(np.asarray(_b)), "kernel assumes LN beta == 0"
    assert np.all(np.asarray(lnf_g) == 1.0) and not np.any(np.asarray(lnf_b))
    common = dict(wq=wq, wk=wk, wv=wv, wp=wp, w1=w1, w2=w2)
    masks = [_make_mask(0), _make_mask(1)]
    cols = [_own_cols(0), _own_cols(1)]
    in_maps = []
    for c in range(N_CORES):
        b = c % B
        rank = c // B
        m = dict(common)
        m["x0"] = np.ascontiguousarray(x0_t[b][:, cols[rank]])
        m["msk"] = masks[rank]
        m["wlm"] = np.ascontiguousarray(wlm_bf[:, rank * NV:(rank + 1) * NV])
        in_maps.append(m)
    return in_maps


def kernel(**inputs):
    nc = _get_nc()
    in_maps = _prep_in_maps(**inputs)
    res = run_bass_kernel_spmd(nc, in_maps, core_ids=list(range(N_CORES)))
    out = np.empty((B, T, V), np.float32)
    for b in range(B):
        lo = res.results[b]["logits"]          # vocab half 0
        hi = res.results[b + B]["logits"]      # vocab half 1
        out[b, :, :NV] = lo
        out[b, :, NV:] = hi[:, :V - NV]
    return out
